# revision 1
# baseline (speedup 1.0000x reference)
"""Multi-head self-attention Trainium2 Bass kernel.

Full-input contract: kernel(**inputs) takes the unsharded inputs
(x [4,2048,1024], Wq [1024,512], bq [512], Wk, bk, Wv [1024,1024], bv)
and returns the full [4,2048,1024] output.

Sharding: 8 cores = 4 batches x 2 head-groups. Core c handles batch c//2
and heads 4*(c%2) .. 4*(c%2)+4. Pure SPMD, no collectives.

Per-core algorithm (N=2048 rows, C=1024, 4 heads, d=64, v=128):
  - load x naturally, PE-transpose into xT (C on partitions)
  - QT/KT = W.T @ xT with head-dim on partitions; V natural (rows on
    partitions); biases folded in via rank-1 (K=1) matmuls
  - scores computed TRANSPOSED: sT[keys,q] = (KT tile).T @ QT, so that
    exp(sT) (ACT, scale fused) is directly the PV rhs operand. No max
    subtraction (|scale*s| < ~4 -> exp safely in fp32 range).
  - msgT[v,q] accumulates over key tiles; row-sums of exp via ones-lhsT
    matmuls (column-tiled 4-wide so 4 run concurrently on the PE array);
    per-q normalization applied after transposing back to natural layout.
"""

import math
import os

import numpy as np

import concourse.bass as bass
import concourse.mybir as mybir
import concourse.tile as tile
from concourse import bacc
from concourse.bass_utils import run_bass_kernel_spmd
from concourse.masks import make_identity

F32 = mybir.dt.float32
BF16 = mybir.dt.bfloat16
F32R = mybir.dt.float32r

# dims
B, N, C = 4, 2048, 1024
QK_DIM, NHEADS = 512, 8
D = QK_DIM // NHEADS          # 64 per-head qk dim
V = 1024 // NHEADS            # 128 per-head value dim
SCALE = 1.0 / math.sqrt(D)
HC = 4                        # heads per core
P = 128
NT = N // P                   # 16 row tiles
CT = C // P                   # 8 contraction tiles
KT = N // P                   # 16 key tiles
QC = 4                        # q chunks of 512
QW = N // QC                  # 512


def build_nc(mode: str = "bf16", repeat: int = 1):
    """Build the per-core Bass program (bf16 matmul operands, fp32 psum)."""
    mmdt = BF16
    mmcast = lambda ap: ap

    nc = bacc.Bacc("TRN2", target_bir_lowering=False, debug=False, num_devices=8)

    x_d = nc.dram_tensor("x", [N, C], F32, kind="ExternalInput").ap()
    wq_d = nc.dram_tensor("wq", [C, HC * D], F32, kind="ExternalInput").ap()
    bq_d = nc.dram_tensor("bq", [HC * D], F32, kind="ExternalInput").ap()
    wk_d = nc.dram_tensor("wk", [C, HC * D], F32, kind="ExternalInput").ap()
    bk_d = nc.dram_tensor("bk", [HC * D], F32, kind="ExternalInput").ap()
    wv_d = nc.dram_tensor("wv", [C, HC * V], F32, kind="ExternalInput").ap()
    bv_d = nc.dram_tensor("bv", [HC * V], F32, kind="ExternalInput").ap()
    out_d = nc.dram_tensor("out", [N, HC * V], F32, kind="ExternalOutput").ap()

    with tile.TileContext(nc) as tc:
      for _rep in range(repeat):
        with tc.tile_pool(name="persist", bufs=1) as persist:
            # persistent SBUF arrays
            xT = [persist.tile([P, N], mmdt, tag=f"xT{ct}", name=f"xT{ct}") for ct in range(CT)]
            QT = [persist.tile([P, N], mmdt, tag=f"QT{hp}", name=f"QT{hp}") for hp in range(2)]
            KTt = [persist.tile([P, N], mmdt, tag=f"KT{hp}", name=f"KT{hp}") for hp in range(2)]
            Vt = [persist.tile([P, HC * V], mmdt, tag=f"V{rt}", name=f"V{rt}") for rt in range(NT)]

            ident = persist.tile([P, P], mmdt, tag="ident")
            make_identity(nc, ident)
            identf = persist.tile([P, P], F32, tag="identf")
            make_identity(nc, identf)

            ones_row = persist.tile([1, QW], mmdt, tag="ones_row")
            nc.vector.memset(ones_row[:], 1.0)
            ones32 = persist.tile([P, 32], mmdt, tag="ones32")
            nc.vector.memset(ones32[:], 1.0)
            inv32 = persist.tile([P, 1], F32, tag="inv32")
            nc.vector.memset(inv32[:], 1.0 / 32.0)

            # biases (as [1, n] rows in matmul dtype)
            bq_sb = persist.tile([1, HC * D], mmdt, tag="bq")
            bk_sb = persist.tile([1, HC * D], mmdt, tag="bk")
            bv_sb = persist.tile([1, HC * V], mmdt, tag="bv")
            # weights in matmul dtype
            wv_sb = [persist.tile([P, HC * V], mmdt, tag=f"wv{ct}", name=f"wv{ct}") for ct in range(CT)]
            wq_sb = [persist.tile([P, HC * D], mmdt, tag=f"wqf{ct}", name=f"wqf{ct}") for ct in range(CT)]
            wk_sb = [persist.tile([P, HC * D], mmdt, tag=f"wkf{ct}", name=f"wkf{ct}") for ct in range(CT)]

            # ---- Phase A: load x (critical path: issue x DMAs first),
            #      transpose into xT ----
            with tc.tile_pool(name="xload", bufs=3) as xload, \
                 tc.tile_pool(name="stage", bufs=3) as stage, \
                 tc.tile_pool(name="tp_psum", bufs=3, space="PSUM") as tp_psum, \
                 tc.tile_pool(name="pp_psum", bufs=3, space="PSUM") as pp_psum:
                xns = []
                for rt in range(NT):
                    xn = xload.tile([P, C], F32, tag="xn", bufs=6, name="xn")
                    if rt in (2, 5):
                        eng = nc.gpsimd  # 3rd queue for the ramp-critical tiles
                    else:
                        eng = nc.sync if rt % 2 == 0 else nc.scalar
                    eng.dma_start(out=xn[:], in_=x_d[rt * P:(rt + 1) * P, :])
                    xns.append(xn)

                # weight/bias staging on a different DMA engine so it does
                # not delay the x loads
                for bd, bs, n_ in ((bq_d, bq_sb, HC * D), (bk_d, bk_sb, HC * D),
                                   (bv_d, bv_sb, HC * V)):
                    st = stage.tile([1, n_], F32, tag="bias_st", name="b_st")
                    nc.gpsimd.dma_start(out=st[:], in_=bd.unsqueeze(0))
                    nc.vector.tensor_copy(bs[:], st[:])
                for ct in range(CT):
                    for wd, ws, n_ in ((wq_d, wq_sb[ct], HC * D),
                                       (wk_d, wk_sb[ct], HC * D),
                                       (wv_d, wv_sb[ct], HC * V)):
                        st = stage.tile([P, n_], F32, tag="w_st", name="w_st")
                        nc.gpsimd.dma_start(out=st[:], in_=wd[ct * P:(ct + 1) * P, :])
                        nc.vector.tensor_copy(ws[:], st[:])

                for rt in range(NT):
                    xn = xns[rt]
                    for ct in range(CT):
                        pt = tp_psum.tile([P, P], F32, name="pt")
                        nc.tensor.transpose(pt[:], xn[:, ct * P:(ct + 1) * P], identf[:])
                        # alternate eviction engine to split the load
                        if ct % 2 == 0:
                            nc.vector.tensor_copy(xT[ct][:, rt * P:(rt + 1) * P], pt[:])
                        else:
                            nc.scalar.copy(xT[ct][:, rt * P:(rt + 1) * P], pt[:])

                # ---- Phases B+C interleaved ----
            # Phase C is ACT(exp)-bound while projections are PE-only, so
            # projection blocks are emitted INTO the attention loop where the
            # PE would otherwise idle. One 2-slot PSUM scratch ring serves
            # proj blocks, sums banks and output transposes (their lifetimes
            # never overlap).
            n_pt_bufs = KT + 2
            with tc.tile_pool(name="sT_psum", bufs=2, space="PSUM") as sT_psum, \
                 tc.tile_pool(name="mT_psum", bufs=2, space="PSUM") as mT_psum, \
                 tc.tile_pool(name="scr_psum", bufs=2, space="PSUM") as scr_psum, \
                 tc.tile_pool(name="pT_pool", bufs=n_pt_bufs) as pT_pool, \
                 tc.tile_pool(name="cwork", bufs=2) as cwork:

                def emit_qk_block(hp, qc, which):
                    w_sb, b_sb, dst = ((wq_sb, bq_sb, QT) if which == 0
                                       else (wk_sb, bk_sb, KTt))
                    ps = scr_psum.tile([P, QW], F32, tag="scr", name="ppqk")
                    for ct in range(CT):
                        nc.tensor.matmul(
                            ps[:],
                            mmcast(w_sb[ct][:, hp * P:(hp + 1) * P]),
                            mmcast(xT[ct][:, qc * QW:(qc + 1) * QW]),
                            start=(ct == 0), stop=False)
                    nc.tensor.matmul(
                        ps[:], mmcast(b_sb[:, hp * P:(hp + 1) * P]),
                        mmcast(ones_row[:]), start=False, stop=True)
                    nc.vector.tensor_copy(dst[hp][:, qc * QW:(qc + 1) * QW], ps[:])

                def emit_v_block(rt):
                    ps = scr_psum.tile([P, HC * V], F32, tag="scr", name="ppv")
                    for ct in range(CT):
                        nc.tensor.matmul(
                            ps[:],
                            mmcast(xT[ct][:, rt * P:(rt + 1) * P]),
                            mmcast(wv_sb[ct][:]),
                            start=(ct == 0), stop=False)
                    nc.tensor.matmul(
                        ps[:], mmcast(ones_row[:, 0:P]),
                        mmcast(bv_sb[:]), start=False, stop=True)
                    nc.vector.tensor_copy(Vt[rt][:], ps[:])

                units = [(hp, qc) for hp in range(2) for qc in range(QC)]
                # pending projection work, consumed during attention loops.
                # Deps: unit (hp,qc) needs Q(hp,qc) at start, K(hp,c) by
                # group 2c (its sT scans ALL key chunks), V[k] by the group
                # whose (pipelined) PV reads k-tile k.
                plan = {u: [] for u in range(len(units))}

                def Q(a, b):
                    return lambda: emit_qk_block(a, b, 0)

                def Kb(a, b):
                    return lambda: emit_qk_block(a, b, 1)

                def Vb(rt):
                    return lambda: emit_v_block(rt)

                upfront = [Q(0, 0), Kb(0, 0),
                           Vb(0), Vb(1), Vb(2), Vb(3)]
                # unit 0: remaining K chunks of hp0 at groups 0/2/4 (needed
                # at 2c), V[4..15] at groups 1..6, Q(0,1) at 7
                for c in range(1, QC):
                    plan[0].append((2 * c - 2, Kb(0, c)))
                for k in range(4, KT):
                    plan[0].append(((k - 4) // 2 + 1, Vb(k)))
                plan[0].append((7, Q(0, 1)))
                plan[1] = [(2, Q(0, 2)), (5, Kb(1, 0))]
                plan[2] = [(2, Q(0, 3)), (5, Kb(1, 1))]
                plan[3] = [(2, Q(1, 0)), (4, Kb(1, 2)), (6, Kb(1, 3))]
                plan[4] = [(2, Q(1, 1))]
                plan[5] = [(2, Q(1, 2))]
                plan[6] = [(2, Q(1, 3))]

                for fn_ in upfront:
                    fn_()

                GS = 2
                NG = KT // GS
                for u, (hp, qc) in enumerate(units):
                    heads = (2 * hp, 2 * hp + 1)  # local head ids
                    qs_ = slice(qc * QW, (qc + 1) * QW)
                    mT = [mT_psum.tile([P, QW], F32, tag="mT", name="mT") for _ in range(2)]
                    pT_slices = [[], []]

                    def emit_pv(g):
                        for i, h in enumerate(heads):
                            for uu in range(GS):
                                kt = GS * g + uu
                                nc.tensor.matmul(
                                    mT[i][:],
                                    mmcast(Vt[kt][:, h * V:(h + 1) * V]),
                                    mmcast(pT_slices[i][kt]),
                                    start=(kt == 0), stop=(kt == KT - 1))

                    for g in range(NG):
                        for i, h in enumerate(heads):
                            po = (h % 2) * D  # partition offset in QT/KT tile
                            sT = sT_psum.tile([P, GS * QW], F32, tag="sT", name="sT")
                            for uu in range(GS):
                                kt = GS * g + uu
                                nc.tensor.matmul(
                                    sT[:, uu * QW:(uu + 1) * QW],
                                    mmcast(KTt[hp][po:po + D, kt * P:(kt + 1) * P]),
                                    mmcast(QT[hp][po:po + D, qs_]),
                                    start=True, stop=True)
                            pT = pT_pool.tile([P, GS * QW], mmdt, tag="pT", name="pT")
                            nc.scalar.activation(
                                pT[:], sT[:],
                                mybir.ActivationFunctionType.Exp, scale=SCALE)
                            pT_slices[i].extend(
                                pT[:, uu * QW:(uu + 1) * QW] for uu in range(GS))
                        for gg, blk in plan[u]:
                            if gg == g:
                                blk()
                        if g > 0:
                            emit_pv(g - 1)
                    emit_pv(NG - 1)

                    # column-tiled sums (4 concurrent M=32 matmuls on col
                    # groups 0/32/64/96; rows replicate the partial sums),
                    # then per-q collapse via a [128x1] (1/32) fp32 matmul.
                    mTs = [cwork.tile([P, QW], BF16, tag="mTs", name="mTs") for _ in range(2)]
                    s4 = [cwork.tile([P, QW], F32, tag="s4", name="s4") for _ in range(2)]
                    for i in range(2):
                        sm = scr_psum.tile([P, QW], F32, tag="scr", name="sm")
                        for r in range(4):
                            for j in range(4):
                                nc.tensor.matmul(
                                    sm[32 * j:32 * (j + 1), :],
                                    mmcast(ones32[:]),
                                    mmcast(pT_slices[i][4 * r + j]),
                                    start=(r == 0), stop=(r == 3),
                                    tile_position=(0, 32 * j),
                                    skip_group_check=True)
                        nc.vector.tensor_copy(s4[i][:], sm[:])
                        nc.vector.tensor_copy(mTs[i][:], mT[i][:])
                    for qs in range(QW // P):
                        for i, h in enumerate(heads):
                            stp = scr_psum.tile([P, P], F32, tag="scr", name="stp")
                            nc.tensor.matmul(
                                stp[:, 0:1], s4[i][:, qs * P:(qs + 1) * P],
                                inv32[:], start=True, stop=True)
                            rcp = cwork.tile([P, 1], F32, tag="rcp")
                            nc.vector.reciprocal(rcp[:], stp[:, 0:1])
                            tail_pool, tail_tag = ((sT_psum, "sT")
                                                   if u == len(units) - 1
                                                   else (scr_psum, "scr"))
                            otp = tail_pool.tile([P, P], BF16, tag=tail_tag, name="otp")
                            nc.tensor.transpose(
                                otp[:], mTs[i][:, qs * P:(qs + 1) * P], ident[:])
                            ob = cwork.tile([P, P], F32, tag="ob")
                            nc.vector.tensor_scalar_mul(ob[:], otp[:], rcp[:])
                            nc.sync.dma_start(
                                out=out_d[qc * QW + qs * P:qc * QW + (qs + 1) * P,
                                          h * V:(h + 1) * V],
                                in_=ob[:])

    nc.compile()
    return nc


_CACHE = {}


def _get_nc(mode: str, repeat: int = 1):
    key = (mode, repeat)
    if key not in _CACHE:
        _CACHE[key] = build_nc(mode, repeat)
    return _CACHE[key]


def make_in_maps(x, Wq, bq, Wk, bk, Wv, bv):
    """Shard full inputs into 8 per-core input maps."""
    x = np.ascontiguousarray(np.asarray(x, dtype=np.float32))
    Wq = np.asarray(Wq, np.float32); bq = np.asarray(bq, np.float32)
    Wk = np.asarray(Wk, np.float32); bk = np.asarray(bk, np.float32)
    Wv = np.asarray(Wv, np.float32); bv = np.asarray(bv, np.float32)
    in_maps = []
    for c in range(8):
        b, g = c // 2, c % 2
        qsl = slice(g * HC * D, (g + 1) * HC * D)
        vsl = slice(g * HC * V, (g + 1) * HC * V)
        in_maps.append({
            "x": np.ascontiguousarray(x[b]),
            "wq": np.ascontiguousarray(Wq[:, qsl]),
            "bq": np.ascontiguousarray(bq[qsl]),
            "wk": np.ascontiguousarray(Wk[:, qsl]),
            "bk": np.ascontiguousarray(bk[qsl]),
            "wv": np.ascontiguousarray(Wv[:, vsl]),
            "bv": np.ascontiguousarray(bv[vsl]),
        })
    return in_maps


def gather_out(results):
    full = np.empty((B, N, 1024), np.float32)
    for c in range(8):
        b, g = c // 2, c % 2
        full[b, :, g * HC * V:(g + 1) * HC * V] = results[c]["out"]
    return full


def kernel(x, Wq, bq, Wk, bk, Wv, bv):
    nc = _get_nc("bf16")
    in_maps = make_in_maps(x, Wq, bq, Wk, bk, Wv, bv)
    res = run_bass_kernel_spmd(nc, in_maps, list(range(8)))
    return gather_out(res.results)



# revision 14
# speedup vs baseline: 1.1482x; 1.1482x over previous
"""Multi-head self-attention Trainium2 Bass kernel (v2).

Full-input contract: kernel(**inputs) takes the unsharded inputs
(x [4,2048,1024], Wq [1024,512], bq [512], Wk, bk, Wv [1024,1024], bv)
and returns the full [4,2048,1024] output.

Sharding: 8 cores = 4 batches x 2 head-groups. Core c handles batch c//2
and heads 4*(c%2) .. 4*(c%2)+4. Pure SPMD, no collectives.

v2 design notes (per core: N=2048 rows, C=1024, 4 heads, d=64, v=128):
  - x and the packed weight tensor are shipped from host in bf16 (matmul
    operand dtype anyway), halving input HBM traffic; biases ride in an
    extra row of the weight tensor. bk is dropped entirely: scores
    (q+bq)@(k+bk) == (q+bq)@k modulo per-query constants, which softmax
    cancels exactly.
  - phase A is pipelined per row tile: load x tile rt, PE-transpose it,
    then immediately emit the K and V projections for key tile rt (their
    contraction only needs that one x tile transposed). Attention on the
    first q chunk starts ~8us in instead of waiting for all of x.
  - scores computed TRANSPOSED: sT[keys,q] = K_tile.T @ Q, exp'd on ACT
    (scale fused) directly into the PV rhs operand. No max subtraction
    (|scale*s| < ~4).
  - softmax denominators: pT tiles are tree-accumulated on DVE (bf16,
    2x mode) and collapsed with a [128x1] ones matmul per q tile --
    removes the 54us of ones-matmul PE work the v1 kernel spent.
  - V bias is applied in the projection (rank-1 matmul); Q bias likewise.
    The final normalize fuses (msg * 1/Z) via tensor_scalar on DVE.
  - optional fp8 (e4m3) DoubleRow paths:
      qk8: Q/K stored fp8 with the per-head d-dim split 32x2 so score
           matmuls run perf_mode=DoubleRow; needs host-permuted W cols.
      pv8: pT (exp output) and V stored fp8; PV and the denominator
           matmuls run DoubleRow over key-tile pairs.
"""

import math
import os

import numpy as np

import concourse.bass as bass
import concourse.mybir as mybir
import concourse.tile as tile
from concourse import bacc
from concourse.alu_op_type import AluOpType
from concourse.bass_utils import run_bass_kernel_spmd
from concourse.masks import make_identity

F32 = mybir.dt.float32
BF16 = mybir.dt.bfloat16
F8 = mybir.dt.float8e4
DR = mybir.MatmulPerfMode.DoubleRow

# dims
B, N, C = 4, 2048, 1024
QK_DIM, NHEADS = 512, 8
D = QK_DIM // NHEADS          # 64 per-head qk dim
V = 1024 // NHEADS            # 128 per-head value dim
SCALE = 1.0 / math.sqrt(D)
HC = 4                        # heads per core
P = 128
NT = N // P                   # 16 row tiles
CT = C // P                   # 8 contraction tiles
KT = N // P                   # 16 key tiles
QC = 4                        # q chunks of 512
QW = N // QC                  # 512
GS = 2                        # key tiles per score/exp group
NG = KT // GS                 # 8 groups


def build_nc_v2(qk8: bool = False, pv8: bool = False, repeat: int = 1):
    pdt = F8 if pv8 else BF16

    nc = bacc.Bacc("TRN2", target_bir_lowering=False, debug=False, num_devices=8)

    x_d = nc.dram_tensor("x", [N, C], BF16, kind="ExternalInput").ap()
    wb_d = nc.dram_tensor("wb", [C + 1, 1024], BF16, kind="ExternalInput").ap()
    out_d = nc.dram_tensor("out", [N, HC * V], F32, kind="ExternalOutput").ap()

    with tile.TileContext(nc) as tc:
      for _rep in range(repeat):
        with tc.tile_pool(name="persist", bufs=1) as persist:
            xT = persist.tile([P, CT, N], BF16, tag="xT", name="xT")
            if qk8:
                Q8 = persist.tile([P, 2, N], F8, tag="Q8", name="Q8")
                K8 = persist.tile([P, 2, N], F8, tag="K8", name="K8")
            else:
                QT = [persist.tile([P, N], BF16, tag=f"QT{hp}", name=f"QT{hp}") for hp in range(2)]
                KTt = [persist.tile([P, N], BF16, tag=f"KT{hp}", name=f"KT{hp}") for hp in range(2)]
            Vt = persist.tile([P, NT, HC * V], pdt, tag="Vt", name="Vt")

            wq_sb = persist.tile([P, CT, HC * D], BF16, tag="wq", name="wq")
            wk_sb = persist.tile([P, CT, HC * D], BF16, tag="wk", name="wk")
            wv_sb = persist.tile([P, CT, HC * V], BF16, tag="wv", name="wv")
            brow = persist.tile([1, 1024], BF16, tag="brow", name="brow")

            ident = persist.tile([P, P], BF16, tag="ident", name="ident")
            make_identity(nc, ident)
            ones_row = persist.tile([1, QW], BF16, tag="ones_row", name="ones_row")
            nc.vector.memset(ones_row[:], 1.0)
            onesP1 = persist.tile([P, 1], BF16, tag="onesP1", name="onesP1")
            nc.vector.memset(onesP1[:], 1.0)
            bvf = persist.tile([P, HC * V], BF16, tag="bvf", name="bvf")
            if pv8:
                ones8 = persist.tile([P, 2, 32], F8, tag="ones8", name="ones8")
                nc.vector.memset(ones8[:], 1.0)
                inv32 = persist.tile([P, 1], F32, tag="inv32", name="inv32")
                nc.vector.memset(inv32[:], 1.0 / 32.0)

            n_pt_bufs = (KT + 2) if pv8 else 6
            with tc.tile_pool(name="xload", bufs=NT) as xload, \
                 tc.tile_pool(name="sT_psum", bufs=2, space="PSUM") as sT_psum, \
                 tc.tile_pool(name="mT_psum", bufs=2, space="PSUM") as mT_psum, \
                 tc.tile_pool(name="scr_psum", bufs=2, space="PSUM") as scr_psum, \
                 tc.tile_pool(name="pT_pool", bufs=n_pt_bufs) as pT_pool, \
                 tc.tile_pool(name="cwork", bufs=4) as cwork:

                # ---- weight + bias DMAs first (needed by the first proj);
                #      all x DMAs issued immediately after, 3 queues ----
                for ct in range(CT):
                    rs = slice(ct * P, (ct + 1) * P)
                    nc.gpsimd.dma_start(out=wq_sb[:, ct, :], in_=wb_d[rs, 0:256])
                    nc.gpsimd.dma_start(out=wk_sb[:, ct, :], in_=wb_d[rs, 256:512])
                    nc.gpsimd.dma_start(out=wv_sb[:, ct, :], in_=wb_d[rs, 512:1024])
                nc.gpsimd.dma_start(out=brow[:], in_=wb_d[C:C + 1, :])

                xns = []
                for rt in range(NT):
                    xn = xload.tile([P, C], BF16, tag="xn", name="xn")
                    eng = (nc.sync, nc.scalar, nc.gpsimd)[rt % 3]
                    eng.dma_start(out=xn[:], in_=x_d[rt * P:(rt + 1) * P, :])
                    xns.append(xn)

                # bv broadcast rows: outer product ones[128] x bv[512]
                psb = scr_psum.tile([P, HC * V], F32, tag="scr", name="bvf_ps")
                nc.tensor.matmul(psb[:], ones_row[:, 0:P], brow[:, 512:1024],
                                 start=True, stop=True)
                nc.vector.tensor_copy(bvf[:], psb[:])

                # ---- phase A blocks (emitted per row tile, interleaved
                #      into the attention loop via `plan`) ----
                def rt_block(rt):
                    xn = xns[rt]
                    # 8 transposes share one PSUM bank = one accumulation
                    # group (bank-granular start/stop), evicted in one copy
                    pt = scr_psum.tile([P, CT, P], BF16, tag="scr", name="pt")
                    for ct in range(CT):
                        nc.tensor.matmul(pt[:, ct, :], xn[:, ct * P:(ct + 1) * P],
                                         ident[:], is_transpose=True,
                                         start=(ct == 0), stop=(ct == CT - 1),
                                         skip_group_check=True)
                    nc.vector.tensor_copy(xT[:, :, rt * P:(rt + 1) * P], pt[:])
                    # K projection for key tile rt (no bk: softmax-invariant)
                    for u in range(2):
                        ps = scr_psum.tile([P, P], F32, tag="scr", name="kps")
                        for ct in range(CT):
                            nc.tensor.matmul(
                                ps[:], wk_sb[:, ct, u * P:(u + 1) * P],
                                xT[:, ct, rt * P:(rt + 1) * P],
                                start=(ct == 0), stop=(ct == CT - 1))
                        if qk8:
                            nc.vector.tensor_copy(K8[:, u, rt * P:(rt + 1) * P], ps[:])
                        else:
                            nc.vector.tensor_copy(KTt[u][:, rt * P:(rt + 1) * P], ps[:])
                    # V projection for row tile rt (bv applied post-normalize,
                    # exact since sum(attn)==1)
                    ps = scr_psum.tile([P, HC * V], F32, tag="scr", name="vps")
                    for ct in range(CT):
                        nc.tensor.matmul(
                            ps[:], xT[:, ct, rt * P:(rt + 1) * P], wv_sb[:, ct, :],
                            start=(ct == 0), stop=(ct == CT - 1))
                    nc.vector.tensor_copy(Vt[:, rt, :], ps[:])

                def q_block(qc):
                    qs_ = slice(qc * QW, (qc + 1) * QW)
                    for u in range(2):
                        ps = scr_psum.tile([P, QW], F32, tag="scr", name="qps")
                        for ct in range(CT):
                            nc.tensor.matmul(
                                ps[:], wq_sb[:, ct, u * P:(u + 1) * P],
                                xT[:, ct, qs_],
                                start=(ct == 0), stop=False)
                        nc.tensor.matmul(ps[:], brow[:, u * P:(u + 1) * P],
                                         ones_row[:], start=False, stop=True)
                        if qk8:
                            nc.vector.tensor_copy(Q8[:, u, qs_], ps[:])
                        else:
                            nc.vector.tensor_copy(QT[u][:, qs_], ps[:])

                for rt in range(4):
                    rt_block(rt)
                q_block(0)

                # pending phase-A work injected into unit group loops:
                # {unit: [(group, thunk), ...]}
                units = [(hp, qc) for hp in range(2) for qc in range(QC)]
                plan = {u: [] for u in range(len(units))}
                for g in range(6):
                    plan[0].append((g, (lambda a: lambda: rt_block(a))(4 + 2 * g)))
                    plan[0].append((g, (lambda a: lambda: rt_block(a))(5 + 2 * g)))
                plan[0].append((6, lambda: q_block(1)))
                plan[1].append((2, lambda: q_block(2)))
                plan[2].append((2, lambda: q_block(3)))

                for u, (hp, qc) in enumerate(units):
                    heads = (2 * hp, 2 * hp + 1)  # local head ids
                    qs_ = slice(qc * QW, (qc + 1) * QW)
                    mT = [mT_psum.tile([P, QW], F32, tag="mT", name="mT")
                          for _ in range(2)]
                    pTs = [[], []]
                    acc = [None, None]

                    def emit_pv(g):
                        for i, h in enumerate(heads):
                            if pv8:
                                nc.tensor.matmul(
                                    mT[i][:],
                                    Vt[:, 2 * g:2 * g + 2, h * V:(h + 1) * V],
                                    pTs[i][g],
                                    start=(g == 0), stop=(g == NG - 1),
                                    perf_mode=DR)
                            else:
                                for uu in range(GS):
                                    kt = GS * g + uu
                                    nc.tensor.matmul(
                                        mT[i][:],
                                        Vt[:, kt, h * V:(h + 1) * V],
                                        pTs[i][g][:, uu, :],
                                        start=(kt == 0), stop=(kt == KT - 1))

                    for g in range(NG):
                        for i, h in enumerate(heads):
                            sT = sT_psum.tile([P, GS * QW], F32, tag="sT", name="sT")
                            for uu in range(GS):
                                kt = GS * g + uu
                                if qk8:
                                    nc.tensor.matmul(
                                        sT[:, uu * QW:(uu + 1) * QW],
                                        K8[32 * h:32 * h + 32, :, kt * P:(kt + 1) * P],
                                        Q8[32 * h:32 * h + 32, :, qs_],
                                        start=True, stop=True, perf_mode=DR)
                                else:
                                    po = (h % 2) * D
                                    nc.tensor.matmul(
                                        sT[:, uu * QW:(uu + 1) * QW],
                                        KTt[hp][po:po + D, kt * P:(kt + 1) * P],
                                        QT[hp][po:po + D, qs_],
                                        start=True, stop=True)
                            pT = pT_pool.tile([P, GS, QW], pdt, tag="pT", name="pT")
                            nc.scalar.activation(
                                pT[:].rearrange("p a b -> p (a b)"), sT[:],
                                mybir.ActivationFunctionType.Exp, scale=SCALE)
                            pTs[i].append(pT)
                            if not pv8:
                                # denominator tree accumulation on DVE (bf16 2x)
                                if g == 0:
                                    a = cwork.tile([P, GS * QW], BF16, tag="acc",
                                                   name="acc")
                                    acc[i] = a
                                    nc.vector.tensor_copy(a[:], pT[:].rearrange("p a b -> p (a b)"))
                                else:
                                    nc.vector.tensor_tensor(
                                        acc[i][:], acc[i][:], pT[:].rearrange("p a b -> p (a b)"),
                                        AluOpType.add)
                        for gg, blk in plan[u]:
                            if gg == g:
                                blk()
                        if g > 0:
                            emit_pv(g - 1)
                    emit_pv(NG - 1)

                    mTs = [cwork.tile([P, QW], BF16, tag="mTs", name="mTs")
                           for _ in range(2)]
                    zps = [None, None]
                    if pv8:
                        s4 = [cwork.tile([P, QW], F32, tag="s4", name="s4")
                              for _ in range(2)]
                        for i in range(2):
                            sm = scr_psum.tile([P, QW], F32, tag="scr", name="sm")
                            for r in range(2):
                                for j in range(4):
                                    nc.tensor.matmul(
                                        sm[32 * j:32 * (j + 1), :],
                                        ones8[:], pTs[i][4 * r + j][:],
                                        start=(r == 0), stop=(r == 1),
                                        perf_mode=DR,
                                        tile_position=(0, 32 * j),
                                        skip_group_check=True)
                            nc.vector.tensor_copy(s4[i][:], sm[:])
                            nc.vector.tensor_copy(mTs[i][:], mT[i][:])
                    else:
                        accf = [cwork.tile([P, QW], BF16, tag="accf", name="accf")
                                for _ in range(2)]
                        for i in range(2):
                            nc.gpsimd.tensor_tensor(
                                accf[i][:], acc[i][:, 0:QW], acc[i][:, QW:2 * QW],
                                AluOpType.add)
                            nc.vector.tensor_copy(mTs[i][:], mT[i][:])

                    for qs in range(QW // P):
                        for i, h in enumerate(heads):
                            tail_pool, tail_tag = ((sT_psum, "sT")
                                                   if u == len(units) - 1
                                                   else (scr_psum, "scr"))
                            stp = scr_psum.tile([P, P], F32, tag="scr", name="stp")
                            if pv8:
                                nc.tensor.matmul(
                                    stp[:, 0:1], s4[i][:, qs * P:(qs + 1) * P],
                                    inv32[:], start=True, stop=True)
                            else:
                                nc.tensor.matmul(
                                    stp[:, 0:1], accf[i][:, qs * P:(qs + 1) * P],
                                    onesP1[:], start=True, stop=True)
                            rcp = cwork.tile([P, 1], F32, tag="rcp", name="rcp")
                            nc.vector.reciprocal(rcp[:], stp[:, 0:1])
                            otp = tail_pool.tile([P, P], BF16, tag=tail_tag,
                                                 name="otp")
                            nc.tensor.transpose(
                                otp[:], mTs[i][:, qs * P:(qs + 1) * P], ident[:])
                            ob = cwork.tile([P, P], F32, tag="ob", name="ob")
                            nc.vector.scalar_tensor_tensor(
                                ob[:], otp[:], rcp[:], bvf[:, h * V:(h + 1) * V],
                                AluOpType.mult, AluOpType.add)
                            nc.sync.dma_start(
                                out=out_d[qc * QW + qs * P:qc * QW + (qs + 1) * P,
                                          h * V:(h + 1) * V],
                                in_=ob[:])

    nc.compile()
    return nc


_CACHE = {}


def _get_nc(mode: str, repeat: int = 1):
    key = (mode, repeat)
    if key not in _CACHE:
        if mode == "v2":
            _CACHE[key] = build_nc_v2(False, False, repeat)
        elif mode == "v2q":
            _CACHE[key] = build_nc_v2(True, False, repeat)
        elif mode == "v2p":
            _CACHE[key] = build_nc_v2(False, True, repeat)
        elif mode == "v2qp":
            _CACHE[key] = build_nc_v2(True, True, repeat)
        else:
            raise ValueError(f"unknown mode {mode}")
    return _CACHE[key]


def _qk_perm():
    """Column permutation for fp8 d-split layout: new[128j+32h+dm] = 64h+32j+dm."""
    perm = np.empty(256, np.int64)
    for j in range(2):
        for h in range(4):
            for dm in range(32):
                perm[128 * j + 32 * h + dm] = 64 * h + 32 * j + dm
    return perm


def make_in_maps(x, Wq, bq, Wk, bk, Wv, bv, mode=None):
    """Shard full inputs into 8 per-core input maps (bf16, packed weights)."""
    import ml_dtypes

    if mode is None:
        mode = KERNEL_MODE

    qk8, _ = {"v2": (False, False), "v2q": (True, False),
              "v2p": (False, True), "v2qp": (True, True)}[mode]
    bf = ml_dtypes.bfloat16
    x = np.asarray(x, np.float32)
    Wq = np.asarray(Wq, np.float32); bq = np.asarray(bq, np.float32)
    Wk = np.asarray(Wk, np.float32)
    Wv = np.asarray(Wv, np.float32); bv = np.asarray(bv, np.float32)
    perm = _qk_perm() if qk8 else np.arange(256)
    in_maps = []
    for c in range(8):
        b, g = c // 2, c % 2
        qsl = slice(g * HC * D, (g + 1) * HC * D)
        vsl = slice(g * HC * V, (g + 1) * HC * V)
        wb = np.zeros((C + 1, 1024), np.float32)
        wb[:C, 0:256] = Wq[:, qsl][:, perm]
        wb[:C, 256:512] = Wk[:, qsl][:, perm]
        wb[:C, 512:1024] = Wv[:, vsl]
        wb[C, 0:256] = bq[qsl][perm]
        wb[C, 512:1024] = bv[vsl]
        in_maps.append({
            "x": np.ascontiguousarray(x[b]).astype(bf),
            "wb": wb.astype(bf),
        })
    return in_maps


def gather_out(results):
    full = np.empty((B, N, 1024), np.float32)
    for c in range(8):
        b, g = c // 2, c % 2
        full[b, :, g * HC * V:(g + 1) * HC * V] = results[c]["out"]
    return full


KERNEL_MODE = os.environ.get("BASS_ATTN_MODE", "v2")


def kernel(x, Wq, bq, Wk, bk, Wv, bv):
    nc = _get_nc(KERNEL_MODE)
    in_maps = make_in_maps(x, Wq, bq, Wk, bk, Wv, bv, KERNEL_MODE)
    res = run_bass_kernel_spmd(nc, in_maps, list(range(8)))
    return gather_out(res.results)


# revision 16
# speedup vs baseline: 1.2976x; 1.1301x over previous
"""Multi-head self-attention Trainium2 Bass kernel (v2).

Full-input contract: kernel(**inputs) takes the unsharded inputs
(x [4,2048,1024], Wq [1024,512], bq [512], Wk, bk, Wv [1024,1024], bv)
and returns the full [4,2048,1024] output.

Sharding: 8 cores = 4 batches x 2 head-groups. Core c handles batch c//2
and heads 4*(c%2) .. 4*(c%2)+4. Pure SPMD, no collectives.

v2 design notes (per core: N=2048 rows, C=1024, 4 heads, d=64, v=128):
  - x and the packed weight tensor are shipped from host in bf16 (matmul
    operand dtype anyway), halving input HBM traffic; biases ride in an
    extra row of the weight tensor. bk is dropped entirely: scores
    (q+bq)@(k+bk) == (q+bq)@k modulo per-query constants, which softmax
    cancels exactly.
  - phase A is pipelined per row tile: load x tile rt, PE-transpose it,
    then immediately emit the K and V projections for key tile rt (their
    contraction only needs that one x tile transposed). Attention on the
    first q chunk starts ~8us in instead of waiting for all of x.
  - scores computed TRANSPOSED: sT[keys,q] = K_tile.T @ Q, exp'd on ACT
    (scale fused) directly into the PV rhs operand. No max subtraction
    (|scale*s| < ~4).
  - softmax denominators: pT tiles are tree-accumulated on DVE (bf16,
    2x mode) and collapsed with a [128x1] ones matmul per q tile --
    removes the 54us of ones-matmul PE work the v1 kernel spent.
  - V bias is applied in the projection (rank-1 matmul); Q bias likewise.
    The final normalize fuses (msg * 1/Z) via tensor_scalar on DVE.
  - optional fp8 (e4m3) DoubleRow paths:
      qk8: Q/K stored fp8 with the per-head d-dim split 32x2 so score
           matmuls run perf_mode=DoubleRow; needs host-permuted W cols.
      pv8: pT (exp output) and V stored fp8; PV and the denominator
           matmuls run DoubleRow over key-tile pairs.
"""

import math
import os

import numpy as np

import concourse.bass as bass
import concourse.mybir as mybir
import concourse.tile as tile
from concourse import bacc
from concourse.alu_op_type import AluOpType
from concourse.bass_utils import run_bass_kernel_spmd
from concourse.masks import make_identity

F32 = mybir.dt.float32
BF16 = mybir.dt.bfloat16
F8 = mybir.dt.float8e4
DR = mybir.MatmulPerfMode.DoubleRow

# dims
B, N, C = 4, 2048, 1024
QK_DIM, NHEADS = 512, 8
D = QK_DIM // NHEADS          # 64 per-head qk dim
V = 1024 // NHEADS            # 128 per-head value dim
SCALE = 1.0 / math.sqrt(D)
HC = 4                        # heads per core
P = 128
NT = N // P                   # 16 row tiles
CT = C // P                   # 8 contraction tiles
KT = N // P                   # 16 key tiles
QC = 4                        # q chunks of 512
QW = N // QC                  # 512
GS = 2                        # key tiles per score/exp group
NG = KT // GS                 # 8 groups


def build_nc_v2(qk8: bool = False, pv8: bool = False, repeat: int = 1):
    pdt = F8 if pv8 else BF16

    nc = bacc.Bacc("TRN2", target_bir_lowering=False, debug=False, num_devices=8)

    x_d = nc.dram_tensor("x", [N, C], BF16, kind="ExternalInput").ap()
    wb_d = nc.dram_tensor("wb", [C + 1, 1024], BF16, kind="ExternalInput").ap()
    out_d = nc.dram_tensor("out", [N, HC * V], F32, kind="ExternalOutput").ap()

    with tile.TileContext(nc) as tc:
      for _rep in range(repeat):
        with tc.tile_pool(name="persist", bufs=1) as persist:
            xT = persist.tile([P, CT, N], BF16, tag="xT", name="xT")
            if qk8:
                Q8 = [persist.tile([64, 2, N], F8, tag=f"Q8{hp}", name=f"Q8{hp}")
                      for hp in range(2)]
                K8 = [persist.tile([64, 2, N], F8, tag=f"K8{hp}", name=f"K8{hp}")
                      for hp in range(2)]
            else:
                QT = [persist.tile([P, N], BF16, tag=f"QT{hp}", name=f"QT{hp}") for hp in range(2)]
                KTt = [persist.tile([P, N], BF16, tag=f"KT{hp}", name=f"KT{hp}") for hp in range(2)]
            Vt = persist.tile([P, NT, HC * V], pdt, tag="Vt", name="Vt")

            wall = persist.tile([P, CT, 1024], BF16, tag="wall", name="wall")
            brow = persist.tile([1, 1024], BF16, tag="brow", name="brow")

            ident = persist.tile([P, P], BF16, tag="ident", name="ident")
            make_identity(nc, ident)
            ones_row = persist.tile([1, QW], BF16, tag="ones_row", name="ones_row")
            nc.vector.memset(ones_row[:], 1.0)
            onesP1 = persist.tile([P, 1], BF16, tag="onesP1", name="onesP1")
            nc.vector.memset(onesP1[:], 1.0)
            bvf = persist.tile([P, HC * V], BF16, tag="bvf", name="bvf")
            if pv8:
                ones8 = persist.tile([P, 2, 32], F8, tag="ones8", name="ones8")
                nc.vector.memset(ones8[:], 1.0)
                inv32 = persist.tile([P, 1], F32, tag="inv32", name="inv32")
                nc.vector.memset(inv32[:], 1.0 / 32.0)

            n_pt_bufs = (KT + 2) if pv8 else 6
            with tc.tile_pool(name="xload", bufs=NT) as xload, \
                 tc.tile_pool(name="sT_psum", bufs=2, space="PSUM") as sT_psum, \
                 tc.tile_pool(name="mT_psum", bufs=2, space="PSUM") as mT_psum, \
                 tc.tile_pool(name="scr_psum", bufs=2, space="PSUM") as scr_psum, \
                 tc.tile_pool(name="pT_pool", bufs=n_pt_bufs) as pT_pool, \
                 tc.tile_pool(name="cwork", bufs=4) as cwork:

                # ---- weight + bias DMAs first (needed by the first proj);
                #      all x DMAs issued immediately after, 3 queues ----
                for ct in range(CT):
                    rs = slice(ct * P, (ct + 1) * P)
                    nc.gpsimd.dma_start(out=wall[:, ct, :], in_=wb_d[rs, :])
                nc.gpsimd.dma_start(out=brow[:], in_=wb_d[C:C + 1, :])

                xns = []
                for r2 in range(NT // 2):
                    xn2 = xload.tile([P, 2, C], BF16, tag="xn", name="xn")
                    eng = (nc.sync, nc.scalar, nc.gpsimd)[r2 % 3]
                    eng.dma_start(
                        out=xn2[:],
                        in_=x_d[r2 * 2 * P:(r2 + 1) * 2 * P, :].rearrange(
                            "(j p) c -> p j c", p=P))
                    xns.extend([xn2[:, 0, :], xn2[:, 1, :]])

                # bv broadcast rows: outer product ones[128] x bv[512]
                psb = scr_psum.tile([P, HC * V], F32, tag="scr", name="bvf_ps")
                nc.tensor.matmul(psb[:], ones_row[:, 0:P], brow[:, 512:1024],
                                 start=True, stop=True)
                nc.vector.tensor_copy(bvf[:], psb[:])

                # ---- phase A blocks (emitted per row tile, interleaved
                #      into the attention loop via `plan`) ----
                def rt_block(rt):
                    xn = xns[rt]
                    # 8 transposes share one PSUM bank = one accumulation
                    # group (bank-granular start/stop), evicted in one copy
                    pt = scr_psum.tile([P, CT, P], BF16, tag="scr", name="pt")
                    for ct in range(CT):
                        nc.tensor.matmul(pt[:, ct, :], xn[:, ct * P:(ct + 1) * P],
                                         ident[:], is_transpose=True,
                                         start=(ct == 0), stop=(ct == CT - 1),
                                         skip_group_check=True)
                    nc.vector.tensor_copy(xT[:, :, rt * P:(rt + 1) * P], pt[:])
                    # V projection for row tile rt (bv applied post-normalize,
                    # exact since sum(attn)==1)
                    ps = scr_psum.tile([P, HC * V], F32, tag="scr", name="vps")
                    for ct in range(CT):
                        nc.tensor.matmul(
                            ps[:], xT[:, ct, rt * P:(rt + 1) * P], wall[:, ct, 512:1024],
                            start=(ct == 0), stop=(ct == CT - 1))
                    nc.vector.tensor_copy(Vt[:, rt, :], ps[:])

                def k_block(kc):
                    # K projection for key chunk kc (512 keys = x tiles
                    # 4kc..4kc+3); no bk: softmax-invariant
                    ks_ = slice(kc * QW, (kc + 1) * QW)
                    for u in range(2):
                        ps = scr_psum.tile([P, QW], F32, tag="scr", name="kps")
                        for ct in range(CT):
                            nc.tensor.matmul(
                                ps[:], wall[:, ct, 256 + u * P:256 + (u + 1) * P],
                                xT[:, ct, ks_],
                                start=(ct == 0), stop=(ct == CT - 1))
                        if qk8:
                            nc.vector.tensor_copy(K8[0][:, u, ks_], ps[0:64, :])
                            nc.vector.tensor_copy(K8[1][:, u, ks_], ps[64:128, :])
                        else:
                            nc.vector.tensor_copy(KTt[u][:, ks_], ps[:])

                def q_block(qc):
                    qs_ = slice(qc * QW, (qc + 1) * QW)
                    for u in range(2):
                        ps = scr_psum.tile([P, QW], F32, tag="scr", name="qps")
                        for ct in range(CT):
                            nc.tensor.matmul(
                                ps[:], wall[:, ct, u * P:(u + 1) * P],
                                xT[:, ct, qs_],
                                start=(ct == 0), stop=False)
                        nc.tensor.matmul(ps[:], brow[:, u * P:(u + 1) * P],
                                         ones_row[:], start=False, stop=True)
                        if qk8:
                            nc.vector.tensor_copy(Q8[0][:, u, qs_], ps[0:64, :])
                            nc.vector.tensor_copy(Q8[1][:, u, qs_], ps[64:128, :])
                        else:
                            nc.vector.tensor_copy(QT[u][:, qs_], ps[:])

                for rt in range(4):
                    rt_block(rt)
                k_block(0)
                q_block(0)

                # pending phase-A work injected into unit group loops:
                # {unit: [(group, thunk), ...]}
                units = [(hp, qc) for hp in range(2) for qc in range(QC)]
                plan = {u: [] for u in range(len(units))}
                for g in range(6):
                    plan[0].append((g, (lambda a: lambda: rt_block(a))(4 + 2 * g)))
                    plan[0].append((g, (lambda a: lambda: rt_block(a))(5 + 2 * g)))
                plan[0].append((1, lambda: k_block(1)))
                plan[0].append((3, lambda: k_block(2)))
                plan[0].append((5, lambda: k_block(3)))
                plan[0].append((6, lambda: q_block(1)))
                plan[1].append((2, lambda: q_block(2)))
                plan[2].append((2, lambda: q_block(3)))

                for u, (hp, qc) in enumerate(units):
                    heads = (2 * hp, 2 * hp + 1)  # local head ids
                    qs_ = slice(qc * QW, (qc + 1) * QW)
                    mT = [mT_psum.tile([P, QW], F32, tag="mT", name="mT")
                          for _ in range(2)]
                    pTs = [[], []]
                    acc = [None, None]

                    def emit_pv(g):
                        for i, h in enumerate(heads):
                            if pv8:
                                nc.tensor.matmul(
                                    mT[i][:],
                                    Vt[:, 2 * g:2 * g + 2, h * V:(h + 1) * V],
                                    pTs[i][g],
                                    start=(g == 0), stop=(g == NG - 1),
                                    perf_mode=DR)
                            else:
                                for uu in range(GS):
                                    kt = GS * g + uu
                                    nc.tensor.matmul(
                                        mT[i][:],
                                        Vt[:, kt, h * V:(h + 1) * V],
                                        pTs[i][g][:, uu, :],
                                        start=(kt == 0), stop=(kt == KT - 1))

                    for g in range(NG):
                        for i, h in enumerate(heads):
                            sT = sT_psum.tile([P, GS * QW], F32, tag="sT", name="sT")
                            for uu in range(GS):
                                kt = GS * g + uu
                                if qk8:
                                    hb = 32 * (h % 2)
                                    nc.tensor.matmul(
                                        sT[:, uu * QW:(uu + 1) * QW],
                                        K8[hp][hb:hb + 32, :, kt * P:(kt + 1) * P],
                                        Q8[hp][hb:hb + 32, :, qs_],
                                        start=True, stop=True, perf_mode=DR)
                                else:
                                    po = (h % 2) * D
                                    nc.tensor.matmul(
                                        sT[:, uu * QW:(uu + 1) * QW],
                                        KTt[hp][po:po + D, kt * P:(kt + 1) * P],
                                        QT[hp][po:po + D, qs_],
                                        start=True, stop=True)
                            pT = pT_pool.tile([P, GS, QW], pdt, tag="pT", name="pT")
                            nc.scalar.activation(
                                pT[:].rearrange("p a b -> p (a b)"), sT[:],
                                mybir.ActivationFunctionType.Exp, scale=SCALE)
                            pTs[i].append(pT)
                            if not pv8:
                                # denominator tree accumulation on DVE (bf16 2x)
                                if g == 0:
                                    a = cwork.tile([P, GS * QW], BF16, tag="acc",
                                                   name="acc")
                                    acc[i] = a
                                    nc.vector.tensor_copy(a[:], pT[:].rearrange("p a b -> p (a b)"))
                                else:
                                    nc.vector.tensor_tensor(
                                        acc[i][:], acc[i][:], pT[:].rearrange("p a b -> p (a b)"),
                                        AluOpType.add)
                        for gg, blk in plan[u]:
                            if gg == g:
                                blk()
                        if g > 0:
                            emit_pv(g - 1)
                    emit_pv(NG - 1)

                    mTs = [cwork.tile([P, QW], BF16, tag="mTs", name="mTs")
                           for _ in range(2)]
                    zps = [None, None]
                    if pv8:
                        s4 = [cwork.tile([P, QW], F32, tag="s4", name="s4")
                              for _ in range(2)]
                        for i in range(2):
                            sm = scr_psum.tile([P, QW], F32, tag="scr", name="sm")
                            for r in range(2):
                                for j in range(4):
                                    nc.tensor.matmul(
                                        sm[32 * j:32 * (j + 1), :],
                                        ones8[:], pTs[i][4 * r + j][:],
                                        start=(r == 0), stop=(r == 1),
                                        perf_mode=DR,
                                        tile_position=(0, 32 * j),
                                        skip_group_check=True)
                            nc.vector.tensor_copy(s4[i][:], sm[:])
                            nc.vector.tensor_copy(mTs[i][:], mT[i][:])
                    else:
                        accf = [cwork.tile([P, QW], BF16, tag="accf", name="accf")
                                for _ in range(2)]
                        for i in range(2):
                            nc.gpsimd.tensor_tensor(
                                accf[i][:], acc[i][:, 0:QW], acc[i][:, QW:2 * QW],
                                AluOpType.add)
                            nc.vector.tensor_copy(mTs[i][:], mT[i][:])

                    for i, h in enumerate(heads):
                        # Z per q for all 4 q tiles: disjoint columns of one
                        # PSUM bank accumulation group, then one reciprocal
                        stp = scr_psum.tile([P, 4], F32, tag="scr", name="stp")
                        for qs in range(QW // P):
                            src_ = s4[i] if pv8 else accf[i]
                            rhs_ = inv32 if pv8 else onesP1
                            nc.tensor.matmul(
                                stp[:, qs:qs + 1],
                                src_[:, qs * P:(qs + 1) * P], rhs_[:],
                                start=(qs == 0), stop=(qs == 3),
                                skip_group_check=True)
                        rcp4 = cwork.tile([P, 4], F32, tag="rcp", name="rcp")
                        nc.vector.reciprocal(rcp4[:], stp[:])
                        ob4 = cwork.tile([P, 4, P], F32, tag="ob", name="ob")
                        for qs in range(QW // P):
                            tail_pool, tail_tag = ((sT_psum, "sT")
                                                   if u == len(units) - 1
                                                   else (scr_psum, "scr"))
                            otp = tail_pool.tile([P, P], BF16, tag=tail_tag,
                                                 name="otp")
                            nc.tensor.transpose(
                                otp[:], mTs[i][:, qs * P:(qs + 1) * P], ident[:])
                            nc.vector.scalar_tensor_tensor(
                                ob4[:, qs, :], otp[:], rcp4[:, qs:qs + 1],
                                bvf[:, h * V:(h + 1) * V],
                                AluOpType.mult, AluOpType.add)
                        nc.sync.dma_start(
                            out=out_d[qc * QW:(qc + 1) * QW,
                                      h * V:(h + 1) * V].rearrange(
                                          "(qs p) c -> p qs c", p=P),
                            in_=ob4[:])

    nc.compile()
    return nc


_CACHE = {}


def _get_nc(mode: str, repeat: int = 1):
    key = (mode, repeat)
    if key not in _CACHE:
        if mode == "v2":
            _CACHE[key] = build_nc_v2(False, False, repeat)
        elif mode == "v2q":
            _CACHE[key] = build_nc_v2(True, False, repeat)
        elif mode == "v2p":
            _CACHE[key] = build_nc_v2(False, True, repeat)
        elif mode == "v2qp":
            _CACHE[key] = build_nc_v2(True, True, repeat)
        else:
            raise ValueError(f"unknown mode {mode}")
    return _CACHE[key]


def _qk_perm():
    """Column permutation for fp8 d-split layout: new[128j+32h+dm] = 64h+32j+dm."""
    perm = np.empty(256, np.int64)
    for j in range(2):
        for h in range(4):
            for dm in range(32):
                perm[128 * j + 32 * h + dm] = 64 * h + 32 * j + dm
    return perm


def make_in_maps(x, Wq, bq, Wk, bk, Wv, bv, mode=None):
    """Shard full inputs into 8 per-core input maps (bf16, packed weights)."""
    import ml_dtypes

    if mode is None:
        mode = KERNEL_MODE

    qk8, _ = {"v2": (False, False), "v2q": (True, False),
              "v2p": (False, True), "v2qp": (True, True)}[mode]
    bf = ml_dtypes.bfloat16
    x = np.asarray(x, np.float32)
    Wq = np.asarray(Wq, np.float32); bq = np.asarray(bq, np.float32)
    Wk = np.asarray(Wk, np.float32)
    Wv = np.asarray(Wv, np.float32); bv = np.asarray(bv, np.float32)
    perm = _qk_perm() if qk8 else np.arange(256)
    in_maps = []
    for c in range(8):
        b, g = c // 2, c % 2
        qsl = slice(g * HC * D, (g + 1) * HC * D)
        vsl = slice(g * HC * V, (g + 1) * HC * V)
        wb = np.zeros((C + 1, 1024), np.float32)
        wb[:C, 0:256] = Wq[:, qsl][:, perm]
        wb[:C, 256:512] = Wk[:, qsl][:, perm]
        wb[:C, 512:1024] = Wv[:, vsl]
        wb[C, 0:256] = bq[qsl][perm]
        wb[C, 512:1024] = bv[vsl]
        in_maps.append({
            "x": np.ascontiguousarray(x[b]).astype(bf),
            "wb": wb.astype(bf),
        })
    return in_maps


def gather_out(results):
    full = np.empty((B, N, 1024), np.float32)
    for c in range(8):
        b, g = c // 2, c % 2
        full[b, :, g * HC * V:(g + 1) * HC * V] = results[c]["out"]
    return full


KERNEL_MODE = os.environ.get("BASS_ATTN_MODE", "v2")


def kernel(x, Wq, bq, Wk, bk, Wv, bv):
    nc = _get_nc(KERNEL_MODE)
    in_maps = make_in_maps(x, Wq, bq, Wk, bk, Wv, bv, KERNEL_MODE)
    res = run_bass_kernel_spmd(nc, in_maps, list(range(8)))
    return gather_out(res.results)
